# revision 1
# baseline (speedup 1.0000x reference)
"""Trainium2 Bass kernel for nn_LocalBlock (LocallyConnected1D + BatchNorm + ReLU).

Computation (reference):
    y[b,l,f] = relu( (sum_{k,c} x[b,l+k,c] * w[l,k*C+c,f] + bias[l,f]) * inv[f]
                     + (beta[f] - mean[f]*inv[f]) )
    inv = gamma * rsqrt(var + eps)

Sharding: positions (L_out) across 8 cores, 64 positions/core (506 padded to 512).
Weights are the dominant traffic (232 MB total) and are fully partitioned by
this split; x is re-read with a K-1 row halo per core.

Per-core kernel:
  - x slice loaded [B, NX, C] (natural layout), PE-transposed to [C, NX, B]
    once (the contraction runs over C, which must sit on partitions).
  - per output position l: DMA w[l] as [C, K, F]; 7 accumulating fp32 matmuls
    with the WEIGHT chunk stationary (lhsT = w[l,k] [C,F], rhs = xT[:,l+k,:]
    [C,B]) giving psum_T [F, B].
  - BN+bias+ReLU in ONE ScalarE activation: relu(psum_T * inv[f] + d[l,f])
    with per-partition scale/bias (d = bias*inv + beta - mean*inv).
  - PE-transpose the [F, B] result back to [B, F], stage, and DMA out.
"""

import numpy as np

import concourse.bass as bass
import concourse.tile as tile
from concourse import bacc, mybir
from concourse.bass_utils import run_bass_kernel_spmd
from concourse.masks import make_identity

F32 = mybir.dt.float32
AF = mybir.ActivationFunctionType
ALU = mybir.AluOpType

B, L, C, F, K = 128, 512, 128, 128, 7
L_OUT = L - K + 1          # 506
N_CORES = 8
NL = 64                    # output positions per core (8*64 = 512 >= 506)
NX = NL + K - 1            # 70 input rows needed per core
BN_EPS = 1e-3
X_CHUNK = 7                # x-load chunk (10 chunks of 7 rows)
O_CHUNK = 8                # output staging chunk (8 chunks of 8 positions)

_CACHED = None


def build_module(w_bufs=12, mm_bufs=4, tr_bufs=4, t_bufs=3, o_bufs=2):
    nc = bacc.Bacc("TRN2", target_bir_lowering=False, debug=False,
                   num_devices=N_CORES)

    x_d = nc.dram_tensor("x", [B, NX, C], F32, kind="ExternalInput").ap()
    w_d = nc.dram_tensor("w", [NL, K * C, F], F32, kind="ExternalInput").ap()
    bias_d = nc.dram_tensor("bias", [NL, F], F32, kind="ExternalInput").ap()
    gamma_d = nc.dram_tensor("gamma", [F], F32, kind="ExternalInput").ap()
    beta_d = nc.dram_tensor("beta", [F], F32, kind="ExternalInput").ap()
    mean_d = nc.dram_tensor("mmean", [F], F32, kind="ExternalInput").ap()
    var_d = nc.dram_tensor("mvar", [F], F32, kind="ExternalInput").ap()
    y_d = nc.dram_tensor("y", [B, NL, F], F32, kind="ExternalOutput").ap()

    with tile.TileContext(nc) as tc:
        with (
            tc.tile_pool(name="singles", bufs=1) as singles,
            tc.tile_pool(name="xbig", bufs=1) as xbig,
            tc.tile_pool(name="wpool", bufs=w_bufs) as wpool,
            tc.tile_pool(name="tpool", bufs=t_bufs) as tpool,
            tc.tile_pool(name="opool", bufs=o_bufs) as opool,
            tc.tile_pool(name="psum_tr", bufs=tr_bufs, space="PSUM") as psum_tr,
            tc.tile_pool(name="psum_mm", bufs=mm_bufs, space="PSUM") as psum_mm,
        ):
            # ---- leading loads on the SP queue (served strictly in order):
            # bias (gates an early PE transpose), then x chunks 0,1 ----
            n_xc = NX // X_CHUNK
            x_sb = xbig.tile([B, NX, C], F32)
            bias_sb = singles.tile([NL, F], F32)
            nc.sync.dma_start(bias_sb, bias_d)

            def load_x_chunk(t):
                sl = slice(t * X_CHUNK, (t + 1) * X_CHUNK)
                nc.sync.dma_start(x_sb[:, sl, :], x_d[:, sl, :])

            load_x_chunk(0)

            # ---- constants ----
            ident = singles.tile([128, 128], F32)
            make_identity(nc, ident)

            # BN stats loaded directly as columns [F, 1] (tiny transposed DMAs)
            gamma_t = singles.tile([F, 1], F32)
            beta_t = singles.tile([F, 1], F32)
            mean_t = singles.tile([F, 1], F32)
            var_t = singles.tile([F, 1], F32)
            nc.scalar.dma_start(gamma_t, gamma_d[:, None])
            nc.scalar.dma_start(beta_t, beta_d[:, None])
            nc.scalar.dma_start(mean_t, mean_d[:, None])
            nc.scalar.dma_start(var_t, var_d[:, None])

            # inv = gamma * rsqrt(var + eps);  shift = beta - mean * inv
            eps_t = singles.tile([F, 1], F32)
            nc.vector.memset(eps_t, float(BN_EPS))
            sq = singles.tile([F, 1], F32)
            nc.scalar.activation(sq, var_t, AF.Sqrt, bias=eps_t, scale=1.0)
            inv_col = singles.tile([F, 1], F32)
            nc.vector.reciprocal(inv_col, sq)
            nc.vector.tensor_mul(inv_col, inv_col, gamma_t)
            shift_col = singles.tile([F, 1], F32)
            nc.vector.tensor_mul(shift_col, mean_t, inv_col)
            nc.vector.tensor_sub(shift_col, beta_t, shift_col)

            # bias [NL, F] -> biasT [F, NL] via PE transpose, then
            # d[f, l] = biasT * inv + shift  (fused per-partition scalars)
            bT_ps = psum_tr.tile([F, NL], F32, tag="tr")
            nc.tensor.transpose(bT_ps, bias_sb, ident[:NL, :NL])
            d_all = singles.tile([F, NL], F32)
            nc.vector.tensor_scalar(out=d_all, in0=bT_ps, scalar1=inv_col,
                                    scalar2=shift_col, op0=ALU.mult, op1=ALU.add)

            # ---- x transposes are interleaved into the main loop: PE's
            # stream is a static FIFO, so each row's transpose is emitted
            # just before the first matmul group that reads it ----
            xT = xbig.tile([C, NX, B], F32)

            def transpose_row(r):
                pt = psum_tr.tile([C, B], F32, tag="tr")
                nc.tensor.transpose(pt, x_sb[:, r, :], ident)
                nc.vector.tensor_copy(xT[:, r, :], pt)

            for r in range(K - 1):          # rows 0..5 (chunk 0)
                transpose_row(r)

            # ---- main loop over output positions ----
            out_t = None
            for j in range(NL):
                wt = wpool.tile([C, K, F], F32)
                nc.sync.dma_start(wt, w_d[j].rearrange("(k c) f -> c k f", c=C))

                r = j + K - 1               # newly needed x row
                if r % X_CHUNK == X_CHUNK - 1 and (r + 1) // X_CHUNK < n_xc:
                    load_x_chunk((r + 1) // X_CHUNK)  # stay a chunk ahead
                transpose_row(r)

                ps = psum_mm.tile([F, B], F32)
                for k in range(K):
                    nc.tensor.matmul(ps, lhsT=wt[:, k, :], rhs=xT[:, j + k, :],
                                     start=(k == 0), stop=(k == K - 1))
                # t_T = relu(psum * inv[f] + d[f, j])   [F, B]
                tT = tpool.tile([F, B], F32)
                nc.scalar.activation(tT, ps, AF.Relu, bias=d_all[:, j:j + 1],
                                     scale=inv_col)
                # transpose back to [B, F]
                po = psum_tr.tile([B, F], F32, tag="tr")
                nc.tensor.transpose(po, tT, ident)

                if j % O_CHUNK == 0:
                    out_t = opool.tile([B, O_CHUNK, F], F32)
                nc.vector.tensor_copy(out_t[:, j % O_CHUNK, :], po)
                if j % O_CHUNK == O_CHUNK - 1:
                    c0 = j - (O_CHUNK - 1)
                    nc.scalar.dma_start(y_d[:, c0:c0 + O_CHUNK, :], out_t)

    nc.compile()
    return nc


def _get_module():
    global _CACHED
    if _CACHED is None:
        _CACHED = build_module()
    return _CACHED


def shard_inputs(x, kernel, bias, gamma, beta, moving_mean, moving_var):
    """Slice full inputs into 8 per-core input maps (position sharding)."""
    in_maps = []
    for i in range(N_CORES):
        l0 = i * NL
        xs = np.zeros((B, NX, C), np.float32)
        xe = min(l0 + NX, L)
        xs[:, :xe - l0, :] = x[:, l0:xe, :]
        ws = np.zeros((NL, K * C, F), np.float32)
        we = min(l0 + NL, L_OUT)
        ws[:we - l0] = kernel[l0:we]
        bs = np.zeros((NL, F), np.float32)
        bs[:we - l0] = bias[l0:we]
        in_maps.append({
            "x": np.ascontiguousarray(xs),
            "w": ws,
            "bias": bs,
            "gamma": np.ascontiguousarray(gamma, dtype=np.float32),
            "beta": np.ascontiguousarray(beta, dtype=np.float32),
            "mmean": np.ascontiguousarray(moving_mean, dtype=np.float32),
            "mvar": np.ascontiguousarray(moving_var, dtype=np.float32),
        })
    return in_maps


def unshard_output(results):
    y = np.empty((B, L_OUT, F), np.float32)
    for i in range(N_CORES):
        l0 = i * NL
        n = min(NL, L_OUT - l0)
        y[:, l0:l0 + n, :] = results[i]["y"][:, :n, :]
    return y


def kernel(x, kernel, bias, gamma, beta, moving_mean, moving_var):
    nc = _get_module()
    in_maps = shard_inputs(x, kernel, bias, gamma, beta,
                           moving_mean, moving_var)
    res = run_bass_kernel_spmd(nc, in_maps, core_ids=list(range(N_CORES)))
    return unshard_output(res.results)



# revision 4
# speedup vs baseline: 2.2185x; 2.2185x over previous
"""Trainium2 Bass kernel for nn_LocalBlock (LocallyConnected1D + BatchNorm + ReLU).

Computation (reference):
    y[b,l,f] = relu( (sum_{k,c} x[b,l+k,c] * w[l,k*C+c,f] + bias[l,f]) * inv[f]
                     + (beta[f] - mean[f]*inv[f]) )
    inv = gamma * rsqrt(var + eps)

Sharding: positions (L_out) across 8 cores, 64 positions/core (506 padded
to 512).  Weights are the dominant traffic and are fully partitioned by this
split; x is re-read with a K-1 row halo per core.

All heavy lifting that does not need the device is done on the host:
  - BN scale folded into the weights (w' = w * inv[f]) and the per-position
    bias folded to d[l,f] = bias*inv + beta - mean*inv.
  - w', x cast to fp16 (halves DMA traffic; rel-err ~1e-3 << 2e-2 budget).
  - x pre-transposed to [C, NX, B] and w' packed to [C, NL, K, F] so every
    DMA is a fully-contiguous >=512B-per-descriptor transfer and the device
    needs NO transposes at all.

Per-core device kernel (per output position j):
  - 7 accumulating fp16 matmuls: lhsT = w'[:, j, k, :] ([C, F], stationary),
    rhs = xT[:, j+k, :] ([C, B]) -> psum[F, B] fp32.
  - one ScalarE activation: relu(psum + d[:, j]) with per-partition bias,
    writing fp16 straight into the output staging tile [F, 8, B].
  - output DMA'd to HBM as [F, NL, B] fp16; host un-transposes + upcasts.
"""

import numpy as np

import concourse.bass as bass
import concourse.tile as tile
from concourse import bacc, mybir
from concourse.bass_utils import run_bass_kernel_spmd

F32 = mybir.dt.float32
F16 = mybir.dt.float16
AF = mybir.ActivationFunctionType

B, L, C, F, K = 128, 512, 128, 128, 7
L_OUT = L - K + 1          # 506
N_CORES = 8
NL = 64                    # output positions per core (8*64 = 512 >= 506)
NX = NL + K - 1            # 70 input rows needed per core
BN_EPS = 1e-3

# Weight-chunk sizes (positions per DMA).  Small first chunk so compute can
# start early; small final chunks so the compute+store tail after the last
# weight transfer is short.
W_CHUNKS = [4, 8, 8, 8, 8, 8, 8, 8, 2, 1, 1]
assert sum(W_CHUNKS) == NL
# x-transposed row chunks (start, count); issued interleaved with w chunks.
XT_CHUNKS = [(0, 14), (14, 28), (42, 28)]
O_CHUNK = 8                # output staging chunk (8 chunks of 8 positions)

_CACHED = None


def build_module(psum_bufs=8, o_bufs=8):
    nc = bacc.Bacc("TRN2", target_bir_lowering=False, debug=False,
                   num_devices=N_CORES)

    xt_d = nc.dram_tensor("xt", [C, NX, B], F16, kind="ExternalInput").ap()
    w_d = nc.dram_tensor("w", [C, NL, K, F], F16, kind="ExternalInput").ap()
    d_d = nc.dram_tensor("d", [F, 128], F32, kind="ExternalInput").ap()
    y_d = nc.dram_tensor("y", [F, NL, B], F16, kind="ExternalOutput").ap()

    # chunk bookkeeping
    w_starts = []
    s = 0
    for g in W_CHUNKS:
        w_starts.append(s)
        s += g

    with tile.TileContext(nc) as tc:
        with (
            tc.tile_pool(name="singles", bufs=1) as singles,
            tc.tile_pool(name="xbig", bufs=1) as xbig,
            tc.tile_pool(name="wpool", bufs=1) as wpool,
            tc.tile_pool(name="opool", bufs=o_bufs) as opool,
            tc.tile_pool(name="psum_mm", bufs=psum_bufs, space="PSUM") as psum_mm,
        ):
            # ---- DMAs: d on the DVE queue; xt/w interleaved on the SP
            # queue so x rows arrive just ahead of the weights that need
            # them.  All are issued up front; sems gate the compute. ----
            d_sb = singles.tile([F, 128], F32)
            nc.scalar.dma_start(d_sb, d_d)

            xt_sb = xbig.tile([C, NX, B], F16)

            def load_xt(ci):
                r0, n = XT_CHUNKS[ci]
                nc.sync.dma_start(xt_sb[:, r0:r0 + n, :], xt_d[:, r0:r0 + n, :])

            w_tiles = []

            def load_w(ci):
                g = W_CHUNKS[ci]
                wt = wpool.tile([C, g, K, F], F16, tag=f"w{ci}",
                                name=f"w_sb{ci}")
                nc.sync.dma_start(wt, w_d[:, w_starts[ci]:w_starts[ci] + g, :, :])
                w_tiles.append(wt)

            load_xt(0)
            load_w(0)
            load_xt(1)
            load_w(1)
            load_w(2)
            load_xt(2)
            for ci in range(3, len(W_CHUNKS)):
                load_w(ci)

            # ---- main loop over output positions ----
            ci = 0
            out_t = None
            for j in range(NL):
                if j >= w_starts[ci] + W_CHUNKS[ci]:
                    ci += 1
                jj = j - w_starts[ci]
                wt = w_tiles[ci]

                ps = psum_mm.tile([F, B], F32, name=f"ps{j}", tag="ps")
                for k in range(K):
                    nc.tensor.matmul(ps, lhsT=wt[:, jj, k, :],
                                     rhs=xt_sb[:, j + k, :],
                                     start=(k == 0), stop=(k == K - 1))

                if j % O_CHUNK == 0:
                    out_t = opool.tile([F, O_CHUNK, B], F16, name=f"ot{j}", tag="ot")
                # relu(psum + d[:, j]) -> fp16 staging
                nc.scalar.activation(out_t[:, j % O_CHUNK, :], ps, AF.Relu,
                                     bias=d_sb[:, j:j + 1])
                if j % O_CHUNK == O_CHUNK - 1:
                    c0 = j - (O_CHUNK - 1)
                    nc.scalar.dma_start(y_d[:, c0:c0 + O_CHUNK, :], out_t)

    nc.compile()
    return nc


def _get_module():
    global _CACHED
    if _CACHED is None:
        _CACHED = build_module()
    return _CACHED


def shard_inputs(x, kernel, bias, gamma, beta, moving_mean, moving_var):
    """Fold BN on the host, cast to fp16, and pre-transpose into the layouts
    the device kernel consumes (position sharding across 8 cores)."""
    inv = (gamma / np.sqrt(moving_var + BN_EPS)).astype(np.float32)   # [F]
    shift = (beta - moving_mean * inv).astype(np.float32)             # [F]
    w16 = (np.asarray(kernel) * inv[None, None, :]).astype(np.float16)
    d_all = (np.asarray(bias) * inv[None, :] + shift[None, :]).astype(np.float32)
    x16 = np.asarray(x).astype(np.float16)                            # [B, L, C]

    in_maps = []
    for i in range(N_CORES):
        l0 = i * NL
        we = min(l0 + NL, L_OUT)
        n = we - l0
        # w: [n, K*C, F] -> [C, NL, K, F]  (c-major so each position's K*F
        # block is contiguous per partition)
        wc = np.zeros((C, NL, K, F), np.float16)
        wc[:, :n] = w16[l0:we].reshape(n, K, C, F).transpose(2, 0, 1, 3)
        # x: [B, NX, C] slice -> [C, NX, B]
        xe = min(l0 + NX, L)
        xt = np.zeros((C, NX, B), np.float16)
        xt[:, :xe - l0, :] = x16[:, l0:xe, :].transpose(2, 1, 0)
        dd = np.zeros((F, 128), np.float32)
        dd[:, :n] = d_all[l0:we].T
        in_maps.append({
            "xt": np.ascontiguousarray(xt),
            "w": np.ascontiguousarray(wc),
            "d": dd,
        })
    return in_maps


def unshard_output(results):
    y = np.empty((B, L_OUT, F), np.float32)
    for i in range(N_CORES):
        l0 = i * NL
        n = min(NL, L_OUT - l0)
        yc = np.asarray(results[i]["y"])          # [F, NL, B] fp16
        y[:, l0:l0 + n, :] = yc[:, :n, :].transpose(2, 1, 0).astype(np.float32)
    return y


def kernel(x, kernel, bias, gamma, beta, moving_mean, moving_var):
    nc = _get_module()
    in_maps = shard_inputs(x, kernel, bias, gamma, beta,
                           moving_mean, moving_var)
    res = run_bass_kernel_spmd(nc, in_maps, core_ids=list(range(N_CORES)))
    return unshard_output(res.results)
